# revision 5
# baseline (speedup 1.0000x reference)
"""Trainium2 Bass kernel for causal self-attention (B=2, S=2048, D=1024, H=16).

Sharding: 8 cores = 2 (batch) x 4 (head groups of 4 heads) — data parallel on
batch, tensor parallel on heads. Each core computes, for its batch b and its
4 heads (256 of the 1024 model dims):

  qT/kT = Wq_slice^T x^T            transposed layouts [head_dim, seq], fp16
  v     = x Wv_slice                natural layout [seq, head_dim], fp16
  per head pair (2 heads share the 128 partitions):
    scoresT[kv, q] blocks on PE (two row-packed K=64 matmuls),
    exp on ACT (psum -> fp16 sbuf), causal mask multiply on DVE (fp16 2x),
    P^T V + replicated ones-row denominators on PE (col-packed M=64),
    normalize: reciprocal_approx_fast + one tensor_mul.
  oT_partial = Wo_slice^T attnT     [1024, seq] fp32 partial

Host: feeds x^T and fp16 weight slices, sums the 4 partials per batch
(the "all-reduce" of the o-projection), transposes, adds bo.

All matmuls run in fp16 (1 cyc/row on PE, same as bf16) with fp32 PSUM
accumulation; softmax scale 1/sqrt(64) is folded into Wq on the host.
"""

import numpy as np

import concourse.bacc as bacc
import concourse.tile as tile
from concourse import mybir
from concourse.bass_utils import run_bass_kernel_spmd

B, S, D, H = 2, 2048, 1024, 16
HD = D // H          # 64
P = 128
NCORES = 8
GROUPS = 4           # head groups (tensor parallel)
HPG = H // GROUPS    # 4 heads per group
CD = HPG * HD        # 256 local head dims per core
QT = 512             # q tile (matmul free dim)
KT = 128             # kv tile (psum partition dim)
NQT = S // QT        # 4
NKT = S // KT        # 16
KD = D // P          # 8 contraction tiles over the model dim

F32 = mybir.dt.float32
F16 = mybir.dt.float16
EXP = mybir.ActivationFunctionType.Exp

_NC_CACHE = {}


def _build_nc():
    if "nc" in _NC_CACHE:
        return _NC_CACHE["nc"]
    nc = bacc.Bacc()
    xt = nc.declare_dram_parameter("xt", [D, S], F16, isOutput=False)
    wq = nc.declare_dram_parameter("wq", [D, CD], F16, isOutput=False)
    wk = nc.declare_dram_parameter("wk", [D, CD], F16, isOutput=False)
    wv = nc.declare_dram_parameter("wv", [D, CD], F16, isOutput=False)
    wo = nc.declare_dram_parameter("wo", [CD, D], F16, isOutput=False)
    bq = nc.declare_dram_parameter("bq", [CD], F32, isOutput=False)
    bk = nc.declare_dram_parameter("bk", [CD], F32, isOutput=False)
    bv = nc.declare_dram_parameter("bv", [HPG, HD], F32, isOutput=False)
    msk = nc.declare_dram_parameter("msk", [4, P, 2 * QT], F16, isOutput=False)
    ot = nc.declare_dram_parameter("ot", [D, S], F32, isOutput=True)

    import concourse.bass as bass

    with tile.TileContext(nc) as tc:
        with tc.tile_pool(name="consts", bufs=1) as consts, \
             tc.tile_pool(name="work", bufs=3) as work:

            # ---- constant / persistent SBUF tensors ----
            xt_sb = consts.tile([P, KD, S], F16)
            wq_sb = consts.tile([P, KD, CD], F16)
            wk_sb = consts.tile([P, KD, CD], F16)
            wv_sb = consts.tile([P, KD, CD], F16)
            wo_sb = consts.tile([P, 2, D], F16)
            bq_sb = consts.tile([P, 2], F32)
            bk_sb = consts.tile([P, 2], F32)
            bv_sb = consts.tile([P, HPG, HD], F32)
            ones_sb = consts.tile([P, 64], F16)
            msk_sb = consts.tile([P, 4, 2 * QT], F16)
            qT_sb = consts.tile([P, 2, S], F16)
            kT_sb = consts.tile([P, 2, S], F16)
            v_sb = consts.tile([P, NKT, HPG, HD], F16)
            aT_sb = consts.tile([P, 2, NQT, QT], F16)

            nc.sync.dma_start(out=xt_sb,
                              in_=xt[:, :].rearrange("(k p) s -> p k s", p=P))
            nc.sync.dma_start(out=wq_sb,
                              in_=wq[:, :].rearrange("(k p) c -> p k c", p=P))
            nc.sync.dma_start(out=wk_sb,
                              in_=wk[:, :].rearrange("(k p) c -> p k c", p=P))
            nc.sync.dma_start(out=wv_sb,
                              in_=wv[:, :].rearrange("(k p) c -> p k c", p=P))
            nc.sync.dma_start(out=wo_sb,
                              in_=wo[:, :].rearrange("(g p) e -> p g e", p=P))
            nc.sync.dma_start(out=bq_sb, in_=bq[:].rearrange("(m p) -> p m", p=P))
            nc.sync.dma_start(out=bk_sb, in_=bk[:].rearrange("(m p) -> p m", p=P))
            bv_ap = bv[:, :]
            bv_bc = bass.AP(tensor=bv_ap.tensor, offset=bv_ap.offset,
                            ap=[[0, P]] + list(bv_ap.ap))
            nc.gpsimd.dma_start(out=bv_sb, in_=bv_bc)
            nc.sync.dma_start(out=msk_sb,
                              in_=msk[:, :, :].rearrange("r p c -> p r c"))
            nc.vector.memset(ones_sb, 1.0)

            # ---- phase 1a: q/k projections (transposed layout) ----
            ph1 = tc.tile_pool(name="ps_ph1", bufs=4, space="PSUM")
            ps_proj = ph1.__enter__()
            ph1v = tc.tile_pool(name="ps_ph1v", bufs=2, space="PSUM")
            ps_v = ph1v.__enter__()
            for w_sb, b_sb, dst in ((wq_sb, bq_sb, qT_sb), (wk_sb, bk_sb, kT_sb)):
                for mt in range(2):
                    ps_list = [ps_proj.tile([P, QT], F32, tag="proj",
                                            name=f"ps_proj_{nt}")
                               for nt in range(NQT)]
                    for kt in range(KD):
                        lhs = w_sb[:, kt, mt * P:(mt + 1) * P]
                        for nt in range(NQT):
                            nc.tensor.matmul(
                                ps_list[nt], lhs,
                                xt_sb[:, kt, nt * QT:(nt + 1) * QT],
                                start=(kt == 0), stop=(kt == KD - 1))
                    for nt in range(NQT):
                        nc.vector.tensor_scalar_add(
                            dst[:, mt, nt * QT:(nt + 1) * QT], ps_list[nt],
                            b_sb[:, mt:mt + 1])

            # ---- phase 1b: v projection (natural layout, + bias) ----
            for jt in range(NKT):
                ps = ps_v.tile([P, CD], F32, tag="vproj")
                for kt in range(KD):
                    nc.tensor.matmul(
                        ps, xt_sb[:, kt, jt * P:(jt + 1) * P],
                        wv_sb[:, kt, :],
                        start=(kt == 0), stop=(kt == KD - 1))
                nc.vector.tensor_add(
                    v_sb[:, jt, :, :],
                    ps.rearrange("p (h d) -> p h d", h=HPG), bv_sb)

            ph1v.__exit__(None, None, None)
            ph1.__exit__(None, None, None)

            # ---- phase 2: attention per q-tile t and head pair g ----
            ph2s = tc.tile_pool(name="ps_s", bufs=2, space="PSUM")
            ps_s = ph2s.__enter__()
            ph2av = tc.tile_pool(name="ps_av", bufs=1, space="PSUM")
            ps_av = ph2av.__enter__()
            ph2o = tc.tile_pool(name="ps_o", bufs=2, space="PSUM")
            ps_o = ph2o.__enter__()
            for t in range(NQT):
                n_kv = 4 * (t + 1)
                for g in range(2):
                    av = ps_av.tile([P, QT], F32, tag="av")
                    den = ps_av.tile([P, QT], F32, tag="den")
                    for kv in range(n_kv):
                        s = ps_s.tile([P, 2 * QT], F32, tag="s")
                        for idx in range(2):
                            p0 = 64 * idx
                            nc.tensor.matmul(
                                s[:, idx * QT:(idx + 1) * QT],
                                kT_sb[p0:p0 + 64, g, kv * KT:(kv + 1) * KT],
                                qT_sb[p0:p0 + 64, g, t * QT:(t + 1) * QT],
                                start=True, stop=True)
                        p_t = work.tile([P, 2 * QT], F16, tag="pt")
                        nc.scalar.activation(p_t, s, EXP)
                        r = kv - 4 * t
                        if r >= 0:
                            nc.vector.tensor_mul(p_t, p_t, msk_sb[:, r, :])
                        for idx in range(2):
                            h = 2 * g + idx
                            nc.tensor.matmul(
                                av[64 * idx:64 * idx + 64, :],
                                v_sb[:, kv, h, :],
                                p_t[:, idx * QT:(idx + 1) * QT],
                                start=(kv == 0), stop=(kv == n_kv - 1),
                                skip_group_check=True,
                                tile_position=(0, 64 * idx))
                            nc.tensor.matmul(
                                den[64 * idx:64 * idx + 64, :],
                                ones_sb,
                                p_t[:, idx * QT:(idx + 1) * QT],
                                start=(kv == 0), stop=(kv == n_kv - 1),
                                skip_group_check=True,
                                tile_position=(0, 64 * idx))
                    # normalize: aT[:, g, t, :] = av * (1 / den)
                    rc = work.tile([P, QT], F32, tag="rc")
                    nc.vector.reciprocal_approx_fast(rc, den)
                    nc.vector.tensor_mul(aT_sb[:, g, t, :], av, rc)

                # ---- phase 3: o_proj for this q tile ----
                for mt_e in range(D // P):
                    ps = ps_o.tile([P, QT], F32, tag="oproj")
                    for g in range(2):
                        nc.tensor.matmul(
                            ps, wo_sb[:, g, mt_e * P:(mt_e + 1) * P],
                            aT_sb[:, g, t, :],
                            start=(g == 0), stop=(g == 1))
                    ot_t = work.tile([P, QT], F32, tag="ot")
                    nc.vector.tensor_copy(ot_t, ps)
                    nc.sync.dma_start(
                        out=ot[mt_e * P:(mt_e + 1) * P, t * QT:(t + 1) * QT],
                        in_=ot_t)
            ph2o.__exit__(None, None, None)
            ph2av.__exit__(None, None, None)
            ph2s.__exit__(None, None, None)

    nc.compile()
    _NC_CACHE["nc"] = nc
    return nc


def _make_masks():
    # msk[r, p, c] for the 4 diagonal kv offsets r: valid iff p <= (c % 512) - 128 r
    m = np.zeros((4, P, 2 * QT), dtype=np.float16)
    pp = np.arange(P)[:, None]
    cc = np.arange(QT)[None, :]
    for r in range(4):
        half = (pp <= cc - KT * r).astype(np.float16)
        m[r, :, :QT] = half
        m[r, :, QT:] = half
    return m


def _in_maps(x, Wq, bq, Wk, bk, Wv, bv, Wo):
    scale = np.float32(1.0 / np.sqrt(HD))
    masks = _make_masks()
    maps = []
    for core in range(NCORES):
        b, g = divmod(core, GROUPS)
        csl = slice(g * CD, (g + 1) * CD)
        maps.append({
            "xt": np.ascontiguousarray(x[b].T).astype(np.float16),
            "wq": np.ascontiguousarray(Wq[:, csl] * scale).astype(np.float16),
            "wk": np.ascontiguousarray(Wk[:, csl]).astype(np.float16),
            "wv": np.ascontiguousarray(Wv[:, csl]).astype(np.float16),
            "wo": np.ascontiguousarray(Wo[csl, :]).astype(np.float16),
            "bq": np.ascontiguousarray(bq[csl] * scale).astype(np.float32),
            "bk": np.ascontiguousarray(bk[csl]).astype(np.float32),
            "bv": np.ascontiguousarray(bv[csl]).reshape(HPG, HD).astype(np.float32),
            "msk": masks,
        })
    return maps


def kernel_with_results(x, Wq, bq, Wk, bk, Wv, bv, Wo, bo, trace=False):
    nc = _build_nc()
    maps = _in_maps(x, Wq, bq, Wk, bk, Wv, bv, Wo)
    kwargs = {}
    if trace:
        kwargs = dict(trace=True, trace_cores=[0])
    res = run_bass_kernel_spmd(nc, maps, core_ids=list(range(NCORES)), **kwargs)
    out = np.zeros((B, S, D), dtype=np.float32)
    for b in range(B):
        acc = np.zeros((D, S), dtype=np.float32)
        for g in range(GROUPS):
            acc += res.results[b * GROUPS + g]["ot"]
        out[b] = acc.T + np.asarray(bo, dtype=np.float32)[None, :]
    return out, res


def kernel(x, Wq, bq, Wk, bk, Wv, bv, Wo, bo):
    out, _ = kernel_with_results(x, Wq, bq, Wk, bk, Wv, bv, Wo, bo, trace=False)
    return out
